# revision 40
# baseline (speedup 1.0000x reference)
"""Trainium2 Bass kernel for a single-layer attention decoder.

Model (see problem reference): B=8 batches, T=S=512, D=512, V=32000.
  x = embed(trg) + PE
  x = LN(x + SelfAttn(x, causal+pad mask))
  x = LN(x + CrossAttn(x, enc, src-length mask))
  x = LN(x + FFN(x))            # D -> 4D -> D, relu
  logits = x @ fcw + fcb        # [B, T, V]

Distribution over 8 NeuronCores: fully data-parallel over batch (one
batch row per core, body AND vocab projection).  No collectives: each
core streams the full (vocab-padded) fcw from HBM during the fc phase
and writes its own [T, V] logits slab in bf16; the host upcasts to f32
and concatenates over batch.

Host-side work is limited to sharding/layout prep: embedding row gather
+ PE add, mask construction from index tensors, dtype casts, weight
folds (1/sqrt(D) into wq, LN3 gain into fcw), and shard concatenation.
"""

import math
import sys
import types

import numpy as np
import ml_dtypes

B, T, S, D, V = 8, 512, 512, 512, 32000
N_CORES = 8
P = 128
KT = D // P                # 4 contraction tiles
TT = T // P                # 4 row tiles
JT = (4 * D) // P          # 16 ffn hidden tiles
VCH = 500                  # fc vocab chunk (one psum bank, 500 of 512 used)
VG = 4 * VCH               # fc vocab group (4 psum banks = 2000 cols)
NG = V // VG               # 16 groups
EPS = 1e-5
NEG = -1e9

BF = ml_dtypes.bfloat16

# toggles for the test harness
TRACE = False
TRACE_DIR = None
MODE = "full"
LAST_EXEC_NS = None
LAST_RESULTS = None
LAST_TMPDIR = None

# Use only vanilla ISA ops (no fused tensor_tensor_reduce /
# scalar_tensor_tensor / activation-accum) — some runtimes reject the
# custom fused DVE micro-ops.
SAFE_OPS = True
LAZY_OVERRIDE = None

_cache = {}


def _sinusoidal_pe(length, d):
    pos = np.arange(length)[:, None].astype(np.float32)
    div = np.exp(np.arange(0, d, 2).astype(np.float32) * (-np.log(10000.0) / d))
    pe = np.zeros((length, d), dtype=np.float32)
    pe[:, 0::2] = np.sin(pos * div)
    pe[:, 1::2] = np.cos(pos * div)
    return pe


def _bf(x):
    return np.ascontiguousarray(np.asarray(x, dtype=np.float32).astype(BF))


def _f32(x):
    return np.ascontiguousarray(np.asarray(x, dtype=np.float32))


def _install_ntff_hook():
    """Register the NTFF profiling hook so trace=True works under axon."""
    if "antenv.axon_hooks" in sys.modules:
        return
    try:
        from trn_agent_boot.trn_boot import _ntff_profile_via_ctypes
        from concourse import bass_utils

        hook = _ntff_profile_via_ctypes("/opt/axon/libaxon_pjrt.so")
        mod = types.ModuleType("antenv.axon_hooks")
        mod.get_axon_ntff_profile_hook = lambda: hook
        mod.set_axon_ntff_profile_hook = lambda h: None
        sys.modules["antenv.axon_hooks"] = mod
        bass_utils.upload_artifacts = lambda tmpdir: "local://" + tmpdir
    except Exception:
        pass


def _build(cfg, mode="full"):
    """Build + compile the per-core SPMD graph. cfg keys:
    exact_mask, b1, gb1, gb2, b2  (bools enabling rarely-needed bias paths).
    mode: "full" | "body" (stop after LN3, dump xn3)."""
    from concourse import bacc, tile
    import concourse.mybir as mybir

    F32 = mybir.dt.float32
    BF16 = mybir.dt.bfloat16
    Alu = mybir.AluOpType
    Act = mybir.ActivationFunctionType

    nc = bacc.Bacc("TRN2", target_bir_lowering=False, debug=False)

    x0_d = nc.dram_tensor("x0", [T, D], BF16, kind="ExternalInput")
    x0T_d = nc.dram_tensor("x0T", [D, T], BF16, kind="ExternalInput")
    encT_d = nc.dram_tensor("encT", [D, S], BF16, kind="ExternalInput")
    if cfg["skip_max"]:
        # exp-then-mask softmax: 0/1 masks instead of additive biases
        sm01_d = nc.dram_tensor("self_m01", [T, S], BF16, kind="ExternalInput")
        cm01_d = nc.dram_tensor("cross_m01", [P, S], BF16, kind="ExternalInput")
        sbias_d = cbias_d = None
    else:
        sbias_d = nc.dram_tensor("self_bias", [T, S], BF16, kind="ExternalInput")
        cbias_d = nc.dram_tensor("cross_bias", [P, S], BF16, kind="ExternalInput")
    wnames = ["wq1", "wk1", "wv1", "wq2", "wk2", "wv2"]
    w_d = {nm: nc.dram_tensor(nm, [D, D], BF16, kind="ExternalInput") for nm in wnames}
    w1_d = nc.dram_tensor("w1", [D, 4 * D], BF16, kind="ExternalInput")
    w2_d = nc.dram_tensor("w2", [4 * D, D], BF16, kind="ExternalInput")
    id_d = nc.dram_tensor("ident", [P, P], BF16, kind="ExternalInput")
    # packed per-partition biases: cols 0-3 bq1, 4-7 bk1, 8-11 bq2, 12-15 bk2, 16-31 b1
    biasp_d = nc.dram_tensor("bias_p", [P, 32], F32, kind="ExternalInput")
    # replicated per-free biases: [:,0,:]=bv1, [:,1,:]=bv2
    bvrep_d = nc.dram_tensor("bias_v", [P, 2, D], BF16, kind="ExternalInput")
    smask_d = None
    if cfg["exact_mask"]:
        smask_d = nc.dram_tensor("self_m01", [T, S], BF16, kind="ExternalInput")
        sm01_d = smask_d
    gb_d = {}
    for key, shape in (("gb1", [P, 2, D]), ("gb2", [P, 2, D]), ("b2", [P, D])):
        if cfg[key]:
            gb_d[key] = nc.dram_tensor(key + "_t", shape, BF16, kind="ExternalInput")
    if mode == "body":
        out_d = nc.dram_tensor("out", [T, D], BF16, kind="ExternalOutput")
    else:
        # fc weights pre-tiled for group streaming: [NG, P, KT, VG]
        fcw_d = nc.dram_tensor("fcw", [NG, P, KT, VG], BF16, kind="ExternalInput")
        out_d = nc.dram_tensor("out", [T, V], BF16, kind="ExternalOutput")

    with tile.TileContext(nc) as tc:
        with (
            tc.tile_pool(name="wp", bufs=1) as wp,
            tc.tile_pool(name="big", bufs=10) as big,
            tc.tile_pool(name="hp", bufs=1) as hp,
            tc.tile_pool(name="wk", bufs=3) as wk,
            tc.tile_pool(name="st", bufs=8) as st,
            tc.tile_pool(name="fcx", bufs=2) as fcx,
            tc.tile_pool(name="lsp", bufs=4) as lsp,
        ):
            # --- tiny tensors first so the PE warmup + first matmuls can
            # start while the bulk weight loads stream in.
            ident_sb = wp.tile([P, P], BF16, tag="ident", name="ident")
            nc.sync.dma_start(ident_sb[:], id_d[:, :])
            biasp_sb = wp.tile([P, 32], F32, tag="biasp", name="biasp")
            nc.sync.dma_start(biasp_sb[:], biasp_d[:, :])
            eps_sb = wp.tile([P, 1], F32, tag="eps", name="eps")
            nc.vector.memset(eps_sb[:], EPS)

            def load3(handle, ktiles, free, dtype, name):
                t_ = wp.tile([P, ktiles, free], dtype, tag=name, name=name)
                nc.sync.dma_start(
                    t_[:], handle[:, :].rearrange("(k p) f -> p k f", p=P)
                )
                return t_

            # ordered by first use in the body
            body_sb = {}
            body_sb["x0T"] = load3(x0T_d, KT, T, BF16, "x0T")
            for nm in ("wq1", "wk1"):
                body_sb[nm] = load3(w_d[nm], KT, D, BF16, nm)
            body_sb["x0"] = load3(x0_d, TT, D, BF16, "x0")
            body_sb["wv1"] = load3(w_d["wv1"], KT, D, BF16, "wv1")
            smask_sb = cbias_sb = cm01_sb = None
            if cfg["skip_max"]:
                body_sb["sm01"] = load3(sm01_d, TT, S, BF16, "sm01")
                cm01_sb = wp.tile([P, S], BF16, tag="cm01", name="cm01")
                nc.sync.dma_start(cm01_sb[:], cm01_d[:, :])
            else:
                body_sb["sbias"] = load3(sbias_d, TT, S, BF16, "sbias")
                if cfg["exact_mask"]:
                    smask_sb = load3(smask_d, TT, S, BF16, "smask")
                cbias_sb = wp.tile([P, S], BF16, tag="cbias", name="cbias")
                nc.sync.dma_start(cbias_sb[:], cbias_d[:, :])
            bvrep_sb = wp.tile([P, 2, D], BF16, tag="bvrep", name="bvrep")
            nc.sync.dma_start(bvrep_sb[:], bvrep_d[:, :, :])
            body_sb["encT"] = load3(encT_d, KT, S, BF16, "encT")
            for nm in ("wq2", "wk2", "wv2"):
                body_sb[nm] = load3(w_d[nm], KT, D, BF16, nm)
            body_sb["w1"] = load3(w1_d, KT, 4 * D, BF16, "w1")
            body_sb["w2"] = load3(w2_d, JT, D, BF16, "w2")
            gb_sb = {}
            for key, t_ in gb_d.items():
                sh = [P, 2, D] if key in ("gb1", "gb2") else [P, D]
                gb_sb[key] = wp.tile(sh, BF16, tag=key, name=key)
                nc.sync.dma_start(gb_sb[key][:], t_[(slice(None),) * len(sh)])

            def mm_accum(psum, lhsT_list, rhs_list):
                n = len(lhsT_list)
                for i, (l_, r_) in enumerate(zip(lhsT_list, rhs_list)):
                    nc.tensor.matmul(psum, l_, r_, start=(i == 0), stop=(i == n - 1))

            def softmax_tile(ps_scores, bias_ap, m01_ap, attn_ap, scale_ap=1.0):
                den = st.tile([P, 1], F32, tag="den", name="den")
                rden = st.tile([P, 1], F32, tag="rden", name="rden")
                if cfg["skip_max"]:
                    # scores are O(1) and no row is fully masked: exp straight
                    # from PSUM (exp(-inf) never occurs; masking is the 0/1
                    # multiply afterwards, since exp(s + b01*NEG) = exp(s)*m01).
                    # scale_ap carries the lazily-applied LN rstd of the query.
                    nc.scalar.activation(attn_ap, ps_scores, Act.Exp, scale=scale_ap)
                    nc.vector.tensor_mul(attn_ap, attn_ap, m01_ap)
                else:
                    masked = wk.tile([P, S], F32, tag="masked", name="masked")
                    if m01_ap is not None:
                        tmp = wk.tile([P, S], F32, tag="masktmp", name="masktmp")
                        nc.vector.tensor_mul(tmp[:], ps_scores, m01_ap)
                        src = tmp[:]
                    else:
                        src = ps_scores
                    nc.vector.tensor_add(masked[:], src, bias_ap)
                    negmx = st.tile([P, 1], F32, tag="negmx", name="negmx")
                    nc.vector.reduce_max(
                        negmx[:], masked[:], axis=mybir.AxisListType.X, negate=True
                    )
                    nc.scalar.activation(
                        attn_ap, masked[:], Act.Exp, bias=negmx[:], scale=1.0
                    )
                nc.vector.reduce_sum(den[:], attn_ap, axis=mybir.AxisListType.X)
                nc.vector.reciprocal(rden[:], den[:])
                nc.vector.tensor_scalar_mul(attn_ap, attn_ap, rden[:])

            def ln_stats(xpre_ap, rstd_tag):
                # mean/var in one DVE pass via bn_stats -> (negmean, rstd)
                stats = st.tile([P, 6], F32, tag="bnst", name="bnst")
                nc.vector.bn_stats(stats[:], xpre_ap)
                aggr = st.tile([P, 2], F32, tag="bnag", name="bnag")
                nc.vector.bn_aggr(aggr[:], stats[:])
                negmean = st.tile([P, 1], F32, tag="negmean", name="negmean")
                nc.vector.tensor_scalar_mul(negmean[:], aggr[:, 0:1], -1.0)
                std = st.tile([P, 1], F32, tag="std", name="std")
                nc.scalar.activation(
                    std[:], aggr[:, 1:2], Act.Sqrt, bias=eps_sb[:], scale=1.0
                )
                rstd = st.tile([P, 1], F32, tag=rstd_tag, name=rstd_tag, bufs=4)
                nc.vector.reciprocal(rstd[:], std[:])
                return negmean, rstd

            def layer_norm(ps_in, res_ap, xn_ap, gb_key):
                xpre = wk.tile([P, D], F32, tag="xpre", name="xpre")
                nc.vector.tensor_add(xpre[:], ps_in, res_ap)
                negmean, rstd = ln_stats(xpre[:], "rstd")
                nc.vector.tensor_scalar(
                    xn_ap, xpre[:], negmean[:], rstd[:],
                    op0=Alu.add, op1=Alu.mult,
                )
                if gb_key is not None and cfg[gb_key]:
                    g_ = gb_sb[gb_key]
                    nc.any.tensor_mul(xn_ap, xn_ap, g_[:, 0, :])
                    nc.any.tensor_add(xn_ap, xn_ap, g_[:, 1, :])

            def ln_add(ps_in, res_ap, xpre_ap, pre_scale=None):
                # Lazy layer norm, part 1: just the residual add (optionally
                # scaling the psum by a deferred row factor first).  The
                # xpre (bf16, UN-normalized) feeds the next transposes —
                # downstream matmuls use mean-centered weights.
                if pre_scale is not None:
                    tmp = wk.tile([P, D], F32, tag="psc", name="psc")
                    nc.vector.tensor_scalar_mul(tmp[:], ps_in, pre_scale[:])
                    nc.vector.tensor_add(xpre_ap, tmp[:], res_ap)
                else:
                    nc.vector.tensor_add(xpre_ap, ps_in, res_ap)

            def ln_tail(xpre_ap, xn_ap, rstd_tag):
                # part 2 (emitted AFTER the transposes so the ACT-queue sqrt
                # never head-of-line blocks the transpose copies): stats,
                # rstd, and the off-critical-path normalized value.
                negmean, rstd = ln_stats(xpre_ap, rstd_tag)
                if xn_ap is not None:
                    nc.vector.tensor_scalar(
                        xn_ap, xpre_ap, negmean[:], rstd[:],
                        op0=Alu.add, op1=Alu.mult,
                    )
                return rstd

            with (
                tc.tile_pool(name="ps", bufs=4, space="PSUM") as ps_pool,
                tc.tile_pool(name="pst", bufs=3, space="PSUM") as pst_pool,
            ):
                # --- PE warmup: ~3.4us of identity matmuls so the HAM clock
                # gate reaches 8/8 before the real body matmuls arrive.  The
                # chain feeds the transpose identity, so it cannot be DCE'd:
                # ident2 = (32 * I) * (1/32) == I exactly.
                wm_ps = pst_pool.tile([P, P], F32, tag="wmps", name="wmps", bufs=1)
                for i in range(32):
                    nc.tensor.matmul(
                        wm_ps[:], ident_sb[:], ident_sb[:],
                        start=(i == 0), stop=(i == 31),
                    )
                ident2 = wp.tile([P, P], BF16, tag="ident2", name="ident2")
                nc.vector.tensor_scalar_mul(ident2[:], wm_ps[:], 1.0 / 32.0)

                def project_T(wtile, xtile, name, bcol):
                    # out[:, m, :] over d'-tiles; out[d', t] = sum_d w[d, d'] x[d, t]
                    o_ = big.tile([P, KT, T], BF16, tag="big", name=name)
                    for m in range(KT):
                        psum = ps_pool.tile([P, T], F32, tag="ps", name="ps")
                        mm_accum(
                            psum[:],
                            [wtile[:, k, m * P:(m + 1) * P] for k in range(KT)],
                            [xtile[:, k, :] for k in range(KT)],
                        )
                        nc.any.tensor_scalar_add(
                            o_[:, m, :], psum[:], biasp_sb[:, bcol + m:bcol + m + 1]
                        )
                    return o_

                def project_V(wtile, xtile, name, bv_idx):
                    # out[s, d'] = sum_d x[d, s] * w[d, d']
                    o_ = big.tile([P, TT, D], BF16, tag="big", name=name)
                    for m in range(TT):
                        psum = ps_pool.tile([P, D], F32, tag="ps", name="ps")
                        mm_accum(
                            psum[:],
                            [xtile[:, k, m * P:(m + 1) * P] for k in range(KT)],
                            [wtile[:, k, :] for k in range(KT)],
                        )
                        nc.any.tensor_add(o_[:, m, :], psum[:], bvrep_sb[:, bv_idx, :])
                    return o_

                def transpose_tile(src_ap, dst_ap, par):
                    pst = pst_pool.tile([P, P], BF16, tag="pst", name="pst")
                    nc.tensor.transpose(pst[:], src_ap, ident2[:])
                    if par % 2:
                        nc.scalar.copy(dst_ap, pst[:])
                    else:
                        nc.vector.tensor_copy(dst_ap, pst[:])

                def transpose_m(src, dst, a_, nFree):
                    # transpose the four 128x128 blocks of src m-tile a_ into
                    # dst column-slice a_
                    for b_ in range(nFree):
                        transpose_tile(
                            src[:, a_, b_ * P:(b_ + 1) * P],
                            dst[:, b_, a_ * P:(a_ + 1) * P],
                            a_ + b_,
                        )

                def transpose512(src, nPart, nFree, name):
                    dst = big.tile([P, nFree, nPart * P], BF16, tag="big", name=name)
                    for a_ in range(nPart):
                        transpose_m(src, dst, a_, nFree)
                    return dst

                def self_softmax_args(m):
                    if cfg["skip_max"]:
                        return None, body_sb["sm01"][:, m, :]
                    return body_sb["sbias"][:, m, :], (
                        smask_sb[:, m, :] if cfg["exact_mask"] else None
                    )

                # ---- self attention
                qt = project_T(body_sb["wq1"], body_sb["x0T"], "qt", 0)
                kt1 = project_T(body_sb["wk1"], body_sb["x0T"], "kt1", 4)
                v1 = project_V(body_sb["wv1"], body_sb["x0T"], "v1", 0)
                attn1 = big.tile([P, TT, S], BF16, tag="big", name="attn1")
                for m in range(TT):
                    pss = ps_pool.tile([P, S], F32, tag="ps", name="ps")
                    mm_accum(
                        pss[:],
                        [qt[:, k, m * P:(m + 1) * P] for k in range(KT)],
                        [kt1[:, k, :] for k in range(KT)],
                    )
                    bias_ap, m01_ap = self_softmax_args(m)
                    softmax_tile(pss[:], bias_ap, m01_ap, attn1[:, m, :])
                # encoder-side projections are independent of the softmax
                # chain above — emit them here so the PE has work while the
                # DVE/ACT softmax pipeline drains.
                k2t = project_T(body_sb["wk2"], body_sb["encT"], "k2t", 12)
                v2 = project_V(body_sb["wv2"], body_sb["encT"], "v2", 1)
                lazy = cfg["lazy"]
                # per-m: transpose attn tile m, attn@V, residual add; in lazy
                # mode the xpre transposes depend only on the add, so they
                # interleave here too — keeping the PE dense while the
                # stats/normalize chains drain on DVE/ACT.
                attn1T = big.tile([P, TT, T], BF16, tag="big", name="attn1T")
                x1s = big.tile([P, TT, D], BF16, tag="big", name="x1s")
                xn1 = big.tile([P, TT, D], BF16, tag="big", name="xn1") if lazy \
                    else x1s
                x1T = big.tile([P, KT, T], BF16, tag="big", name="x1T")
                rstd1 = []
                for m in range(TT):
                    transpose_m(attn1, attn1T, m, TT)
                    pso = ps_pool.tile([P, D], F32, tag="ps", name="ps")
                    mm_accum(
                        pso[:],
                        [attn1T[:, s_, m * P:(m + 1) * P] for s_ in range(TT)],
                        [v1[:, s_, :] for s_ in range(TT)],
                    )
                    if lazy:
                        ln_add(pso[:], body_sb["x0"][:, m, :], x1s[:, m, :])
                        rstd1.append(ln_tail(
                            x1s[:, m, :], xn1[:, m, :], "rstd1"))
                        transpose_m(x1s, x1T, m, KT)
                    else:
                        layer_norm(pso[:], body_sb["x0"][:, m, :],
                                   x1s[:, m, :], "gb1")
                if not lazy:
                    for m in range(TT):
                        transpose_m(x1s, x1T, m, KT)

                # ---- cross attention
                q2t = project_T(body_sb["wq2"], x1T, "q2t", 8)
                attn2 = big.tile([P, TT, S], BF16, tag="big", name="attn2")
                for m in range(TT):
                    pss = ps_pool.tile([P, S], F32, tag="ps", name="ps")
                    mm_accum(
                        pss[:],
                        [q2t[:, k, m * P:(m + 1) * P] for k in range(KT)],
                        [k2t[:, k, :] for k in range(KT)],
                    )
                    scl = rstd1[m][:] if lazy else 1.0
                    if cfg["skip_max"]:
                        softmax_tile(pss[:], None, cm01_sb[:, :], attn2[:, m, :],
                                     scale_ap=scl)
                    else:
                        softmax_tile(pss[:], cbias_sb[:, :], None, attn2[:, m, :])
                attn2T = big.tile([P, TT, T], BF16, tag="big", name="attn2T")
                x2s = big.tile([P, TT, D], BF16, tag="big", name="x2s")
                xn2 = big.tile([P, TT, D], BF16, tag="big", name="xn2") if lazy \
                    else x2s
                x2T = big.tile([P, KT, T], BF16, tag="big", name="x2T")
                rstd2 = []
                for m in range(TT):
                    transpose_m(attn2, attn2T, m, TT)
                    pso = ps_pool.tile([P, D], F32, tag="ps", name="ps")
                    mm_accum(
                        pso[:],
                        [attn2T[:, s_, m * P:(m + 1) * P] for s_ in range(TT)],
                        [v2[:, s_, :] for s_ in range(TT)],
                    )
                    if lazy:
                        ln_add(pso[:], xn1[:, m, :], x2s[:, m, :])
                        rstd2.append(ln_tail(
                            x2s[:, m, :], xn2[:, m, :], "rstd2"))
                        transpose_m(x2s, x2T, m, KT)
                    else:
                        layer_norm(pso[:], x1s[:, m, :], x2s[:, m, :], "gb2")
                if not lazy:
                    for m in range(TT):
                        transpose_m(x2s, x2T, m, KT)

                # ---- FFN
                if cfg["b2"]:
                    x2r = big.tile([P, TT, D], BF16, tag="big", name="x2r")
                    for m in range(TT):
                        nc.any.tensor_add(x2r[:, m, :], xn2[:, m, :],
                                          gb_sb["b2"][:, :])
                else:
                    x2r = xn2
                hT = hp.tile([P, JT, T], BF16, tag="hT", name="hT")
                for j in range(JT):
                    psh = ps_pool.tile([P, T], F32, tag="ps", name="ps")
                    mm_accum(
                        psh[:],
                        [body_sb["w1"][:, k, j * P:(j + 1) * P] for k in range(KT)],
                        [x2T[:, k, :] for k in range(KT)],
                    )
                    if not cfg["b1"]:
                        nc.any.tensor_scalar_max(hT[:, j, :], psh[:], 0.0)
                    else:
                        hb = wk.tile([P, T], F32, tag="hb", name="hb")
                        nc.any.tensor_scalar_add(
                            hb[:], psh[:], biasp_sb[:, 16 + j:16 + j + 1]
                        )
                        nc.any.tensor_scalar_max(hT[:, j, :], hb[:], 0.0)
                x3s = big.tile([P, TT, D], BF16, tag="big", name="x3s")
                x3T = None
                if mode != "body":
                    x3T = big.tile([P, KT, T], BF16, tag="big", name="x3T")
                rstd3 = []
                for m in range(TT):
                    psy = ps_pool.tile([P, D], F32, tag="ps", name="ps")
                    mm_accum(
                        psy[:],
                        [hT[:, j, m * P:(m + 1) * P] for j in range(JT)],
                        [body_sb["w2"][:, j, :] for j in range(JT)],
                    )
                    if lazy:
                        # relu output was computed from un-normalized x2pre:
                        # scale the FFN2 psum by rstd2 (relu is positively
                        # homogeneous) before adding the xn2 residual.
                        if mode == "body":
                            xpre3_ap = wk.tile(
                                [P, D], BF16, tag="xp3", name="xp3")[:]
                            xn3_ap = x3s[:, m, :]
                        else:
                            xpre3_ap = x3s[:, m, :]
                            xn3_ap = None
                        ln_add(psy[:], x2r[:, m, :], xpre3_ap,
                               pre_scale=rstd2[m])
                        rstd3.append(ln_tail(xpre3_ap, xn3_ap, "rstd3"))
                        if x3T is not None:
                            transpose_m(x3s, x3T, m, KT)
                    else:
                        layer_norm(psy[:], x2r[:, m, :], x3s[:, m, :], None)
                        if x3T is not None:
                            transpose_m(x3s, x3T, m, KT)
                xn3 = x3s

            if mode == "body":
                nc.sync.dma_start(
                    out_d[:, :].rearrange("(m p) d -> p m d", p=P), xn3[:]
                )
            else:
                # ---- fc: stream fcw groups, x3T stationary.
                # psum tile = 4 full banks [P, 4, 512]; all 2048 cols of a
                # group evacuate as one contiguous copy + one DMA.
                with tc.tile_pool(name="psfc", bufs=2, space="PSUM") as psfc_pool:
                    for g in range(NG):
                        fcg = fcx.tile([P, KT, VG], BF16, tag="fcg", name="fcg")
                        nc.sync.dma_start(fcg[:], fcw_d[g, :, :, :])
                        for m in range(TT):
                            psl = psfc_pool.tile(
                                [P, 4, 512], F32, tag="psl", name="psl"
                            )
                            for k in range(KT):
                                for v in range(4):
                                    nc.tensor.matmul(
                                        psl[:, v, 0:VCH],
                                        x3T[:, k, m * P:(m + 1) * P],
                                        fcg[:, k, v * VCH:(v + 1) * VCH],
                                        start=(k == 0), stop=(k == KT - 1),
                                    )
                            # evacuate as two halves on both DVE and ACT to
                            # halve the drain latency and balance the engines;
                            # in lazy mode this also applies the LN3 rstd.
                            lsb = lsp.tile([P, VG], BF16, tag="lsb", name="lsb")
                            if cfg["lazy"]:
                                nc.vector.tensor_scalar_mul(
                                    lsb[:, 0:2 * VCH], psl[:, 0:2, 0:VCH],
                                    rstd3[m][:],
                                )
                                nc.scalar.mul(
                                    lsb[:, 2 * VCH:VG], psl[:, 2:4, 0:VCH],
                                    rstd3[m][:],
                                )
                            else:
                                nc.vector.tensor_copy(
                                    lsb[:, 0:2 * VCH], psl[:, 0:2, 0:VCH]
                                )
                                nc.scalar.copy(
                                    lsb[:, 2 * VCH:VG], psl[:, 2:4, 0:VCH]
                                )
                            nc.sync.dma_start(
                                out_d[m * P:(m + 1) * P, g * VG:(g + 1) * VG],
                                lsb[:],
                            )

    nc.compile()
    return nc


def _host_prep(inputs):
    """Shared host-side prep: returns (cfg, in_maps)."""
    trg = np.asarray(inputs["trg_input"])
    enc = _f32(inputs["encoder_hiddens"])
    src_len = np.asarray(inputs["src_lengths"])
    emb = _f32(inputs["embedding"])
    g = {k: _f32(inputs[k]) for k in (
        "wq1", "bq1", "wk1", "bk1", "wv1", "bv1",
        "wq2", "bq2", "wk2", "bk2", "wv2", "bv2",
        "w1", "b1", "w2", "b2", "fcw", "fcb",
        "g1", "be1", "g2", "be2", "g3", "be3")}

    scale = 1.0 / math.sqrt(float(D))
    pe = _sinusoidal_pe(T, D)
    x0 = emb[trg] + pe[None]                      # [B, T, D] f32

    causal = np.tril(np.ones((T, T), dtype=bool))
    pad = trg != 0                                 # [B, T]
    self_mask = pad[:, None, :] & causal[None]     # [B, T, T]
    self_bias = np.where(self_mask, 0.0, NEG).astype(np.float32)
    exact_mask = bool((~self_mask).all(axis=2).any())

    sidx = np.arange(S)[None, :] < src_len[:, None]   # [B, S]
    cross_bias = np.where(sidx, 0.0, NEG).astype(np.float32)

    fcw_eff = g["g3"][:, None] * g["fcw"]
    fcb_eff = g["be3"] @ g["fcw"] + g["fcb"]

    cfg = {
        "exact_mask": exact_mask,
        "skip_max": not exact_mask,
        "b1": bool((g["b1"] != 0.0).any()),
        "gb1": bool((g["g1"] != 1.0).any() or (g["be1"] != 0.0).any()),
        "gb2": bool((g["g2"] != 1.0).any() or (g["be2"] != 0.0).any()),
        "b2": bool((g["b2"] != 0.0).any()),
    }
    # lazy layer-norm folding: mean subtraction is folded into column-centered
    # downstream weights; rstd is applied as a late row scale.  Requires the
    # plain-softmax path and no LN gains / q2 / ffn1 biases.
    cfg["lazy"] = bool(
        cfg["skip_max"] and not cfg["b1"] and not cfg["gb1"] and not cfg["gb2"]
        and not (g["bq2"] != 0.0).any()
    )
    if LAZY_OVERRIDE is not None:
        cfg["lazy"] = bool(LAZY_OVERRIDE) and cfg["lazy"]

    bias_p = np.zeros((P, 32), dtype=np.float32)
    bias_p[:, 0:4] = (g["bq1"] * scale).reshape(KT, P).T
    bias_p[:, 4:8] = g["bk1"].reshape(KT, P).T
    bias_p[:, 8:12] = (g["bq2"] * scale).reshape(KT, P).T
    bias_p[:, 12:16] = g["bk2"].reshape(KT, P).T
    bias_p[:, 16:32] = g["b1"].reshape(JT, P).T
    bias_v = np.stack(
        [np.broadcast_to(g["bv1"], (P, D)), np.broadcast_to(g["bv2"], (P, D))],
        axis=1,
    )

    wq2_eff = g["wq2"] * scale
    w1_eff = g["w1"]
    if cfg["lazy"]:
        # center the columns: x_centered @ w == x @ (w - colmean(w)) for
        # rows x of any mean, because sum_d (x_d - mu) w_d = x@(w - mean)
        # + mu*0 ... exactly folds the LN mean subtraction into the weight.
        wq2_eff = wq2_eff - wq2_eff.mean(axis=0, keepdims=True)
        w1_eff = w1_eff - w1_eff.mean(axis=0, keepdims=True)
        fcw_eff = fcw_eff - fcw_eff.mean(axis=0, keepdims=True)

    # fcw pre-tiled for streaming: [NG, P, KT, VG]
    fcw_t = _bf(fcw_eff.reshape(KT, P, NG, VG).transpose(2, 1, 0, 3))

    shared = {
        "wq1": _bf(g["wq1"] * scale), "wk1": _bf(g["wk1"]), "wv1": _bf(g["wv1"]),
        "wq2": _bf(wq2_eff), "wk2": _bf(g["wk2"]), "wv2": _bf(g["wv2"]),
        "w1": _bf(w1_eff), "w2": _bf(g["w2"]),
        "ident": _bf(np.eye(P, dtype=np.float32)),
        "bias_p": bias_p, "bias_v": _bf(bias_v),
        "fcw": fcw_t,
    }
    if cfg["gb1"]:
        shared["gb1_t"] = _bf(np.stack(
            [np.broadcast_to(g["g1"], (P, D)), np.broadcast_to(g["be1"], (P, D))], 1))
    if cfg["gb2"]:
        shared["gb2_t"] = _bf(np.stack(
            [np.broadcast_to(g["g2"], (P, D)), np.broadcast_to(g["be2"], (P, D))], 1))
    if cfg["b2"]:
        shared["b2_t"] = _bf(np.broadcast_to(g["b2"], (P, D)))

    in_maps = []
    for c in range(N_CORES):
        m = dict(shared)
        m["x0"] = _bf(x0[c])
        m["x0T"] = _bf(x0[c].T)
        m["encT"] = _bf(enc[c].T)
        if cfg["skip_max"]:
            m["self_m01"] = _bf(self_mask[c].astype(np.float32))
            m["cross_m01"] = _bf(
                np.broadcast_to(sidx[c].astype(np.float32), (P, S)))
        else:
            m["self_bias"] = _bf(self_bias[c])
            m["cross_bias"] = _bf(np.broadcast_to(cross_bias[c], (P, S)))
            if cfg["exact_mask"]:
                m["self_m01"] = _bf(self_mask[c].astype(np.float32))
        in_maps.append(m)
    return cfg, in_maps, fcb_eff


def _filter_in_maps(nc, in_maps):
    """Keep only the dram parameters this graph actually declares."""
    import concourse.mybir as mybir

    declared = set()
    for alloc in nc.m.functions[0].allocations:
        if isinstance(alloc, mybir.MemoryLocationSet) and alloc.kind == "ExternalInput":
            declared.add(alloc.memorylocations[0].name)
    return [{k: v for k, v in m.items() if k in declared} for m in in_maps]


def _run(nc, in_maps):
    global LAST_EXEC_NS, LAST_RESULTS, LAST_TMPDIR
    from concourse import bass_utils

    # Warm up the PJRT backend with a trivial op first — the bass custom-call
    # as the very first program has been observed to stall device init.
    import jax
    import jax.numpy as jnp

    jnp.add(
        jax.device_put(np.ones((8, 8), np.float32), jax.devices()[0]), 1.0
    ).block_until_ready()

    kwargs = {}
    if TRACE:
        _install_ntff_hook()
        kwargs = {"trace": True}
        if TRACE_DIR:
            import tempfile

            kwargs["tmpdir"] = tempfile.mkdtemp(prefix="run_", dir=TRACE_DIR)
            LAST_TMPDIR = kwargs["tmpdir"]
    res = bass_utils.run_bass_kernel_spmd(
        nc, _filter_in_maps(nc, in_maps), core_ids=list(range(N_CORES)), **kwargs
    )
    LAST_EXEC_NS = res.exec_time_ns
    LAST_RESULTS = res
    return res


def kernel(**inputs):
    cfg, in_maps, fcb_eff = _host_prep(inputs)
    key = (MODE,) + tuple(sorted(cfg.items()))
    if key not in _cache:
        _cache[key] = _build(cfg, MODE)
    nc = _cache[key]
    res = _run(nc, in_maps)
    if MODE != "full":
        return [np.asarray(res.results[c]["out"]) for c in range(N_CORES)]
    out = np.stack(
        [
            np.asarray(res.results[c]["out"]).astype(np.float32)
            for c in range(N_CORES)
        ],
        axis=0,
    )
    if fcb_eff.any():
        out += fcb_eff[None, None, :]
    return out


# revision 41
# speedup vs baseline: 1.2121x; 1.2121x over previous
"""Trainium2 Bass kernel for a single-layer attention decoder.

Model (see problem reference): B=8 batches, T=S=512, D=512, V=32000.
  x = embed(trg) + PE
  x = LN(x + SelfAttn(x, causal+pad mask))
  x = LN(x + CrossAttn(x, enc, src-length mask))
  x = LN(x + FFN(x))            # D -> 4D -> D, relu
  logits = x @ fcw + fcb        # [B, T, V]

Distribution over 8 NeuronCores: fully data-parallel over batch (one
batch row per core, body AND vocab projection).  No collectives: each
core streams the full (vocab-padded) fcw from HBM during the fc phase
and writes its own [T, V] logits slab in bf16; the host upcasts to f32
and concatenates over batch.

Host-side work is limited to sharding/layout prep: embedding row gather
+ PE add, mask construction from index tensors, dtype casts, weight
folds (1/sqrt(D) into wq, LN3 gain into fcw), and shard concatenation.
"""

import math
import sys
import types

import numpy as np
import ml_dtypes

B, T, S, D, V = 8, 512, 512, 512, 32000
N_CORES = 8
P = 128
KT = D // P                # 4 contraction tiles
TT = T // P                # 4 row tiles
JT = (4 * D) // P          # 16 ffn hidden tiles
VCH = 500                  # fc vocab chunk (one psum bank, 500 of 512 used)
VG = 4 * VCH               # fc vocab group (4 psum banks = 2000 cols)
NG = V // VG               # 16 groups
EPS = 1e-5
NEG = -1e9

BF = ml_dtypes.bfloat16

# toggles for the test harness
TRACE = False
TRACE_DIR = None
MODE = "full"
LAST_EXEC_NS = None
LAST_RESULTS = None
LAST_TMPDIR = None

# Use only vanilla ISA ops (no fused tensor_tensor_reduce /
# scalar_tensor_tensor / activation-accum) — some runtimes reject the
# custom fused DVE micro-ops.
SAFE_OPS = True
LAZY_OVERRIDE = None

_cache = {}


def _sinusoidal_pe(length, d):
    pos = np.arange(length)[:, None].astype(np.float32)
    div = np.exp(np.arange(0, d, 2).astype(np.float32) * (-np.log(10000.0) / d))
    pe = np.zeros((length, d), dtype=np.float32)
    pe[:, 0::2] = np.sin(pos * div)
    pe[:, 1::2] = np.cos(pos * div)
    return pe


def _bf(x):
    return np.ascontiguousarray(np.asarray(x, dtype=np.float32).astype(BF))


def _f32(x):
    return np.ascontiguousarray(np.asarray(x, dtype=np.float32))


def _install_ntff_hook():
    """Register the NTFF profiling hook so trace=True works under axon."""
    if "antenv.axon_hooks" in sys.modules:
        return
    try:
        from trn_agent_boot.trn_boot import _ntff_profile_via_ctypes
        from concourse import bass_utils

        hook = _ntff_profile_via_ctypes("/opt/axon/libaxon_pjrt.so")
        mod = types.ModuleType("antenv.axon_hooks")
        mod.get_axon_ntff_profile_hook = lambda: hook
        mod.set_axon_ntff_profile_hook = lambda h: None
        sys.modules["antenv.axon_hooks"] = mod
        bass_utils.upload_artifacts = lambda tmpdir: "local://" + tmpdir
    except Exception:
        pass


def _build(cfg, mode="full"):
    """Build + compile the per-core SPMD graph. cfg keys:
    exact_mask, b1, gb1, gb2, b2  (bools enabling rarely-needed bias paths).
    mode: "full" | "body" (stop after LN3, dump xn3)."""
    from concourse import bacc, tile
    import concourse.mybir as mybir

    F32 = mybir.dt.float32
    BF16 = mybir.dt.bfloat16
    Alu = mybir.AluOpType
    Act = mybir.ActivationFunctionType

    nc = bacc.Bacc("TRN2", target_bir_lowering=False, debug=False)

    x0_d = nc.dram_tensor("x0", [T, D], BF16, kind="ExternalInput")
    x0T_d = nc.dram_tensor("x0T", [D, T], BF16, kind="ExternalInput")
    encT_d = nc.dram_tensor("encT", [D, S], BF16, kind="ExternalInput")
    if cfg["skip_max"]:
        # exp-then-mask softmax: 0/1 masks instead of additive biases
        sm01_d = nc.dram_tensor("self_m01", [T, S], BF16, kind="ExternalInput")
        cm01_d = nc.dram_tensor("cross_m01", [P, S], BF16, kind="ExternalInput")
        sbias_d = cbias_d = None
    else:
        sbias_d = nc.dram_tensor("self_bias", [T, S], BF16, kind="ExternalInput")
        cbias_d = nc.dram_tensor("cross_bias", [P, S], BF16, kind="ExternalInput")
    wnames = ["wq1", "wk1", "wv1", "wq2", "wk2", "wv2"]
    w_d = {nm: nc.dram_tensor(nm, [D, D], BF16, kind="ExternalInput") for nm in wnames}
    w1_d = nc.dram_tensor("w1", [D, 4 * D], BF16, kind="ExternalInput")
    w2_d = nc.dram_tensor("w2", [4 * D, D], BF16, kind="ExternalInput")
    id_d = nc.dram_tensor("ident", [P, P], BF16, kind="ExternalInput")
    # packed per-partition biases: cols 0-3 bq1, 4-7 bk1, 8-11 bq2, 12-15 bk2, 16-31 b1
    biasp_d = nc.dram_tensor("bias_p", [P, 32], F32, kind="ExternalInput")
    # replicated per-free biases: [:,0,:]=bv1, [:,1,:]=bv2
    bvrep_d = nc.dram_tensor("bias_v", [P, 2, D], BF16, kind="ExternalInput")
    smask_d = None
    if cfg["exact_mask"]:
        smask_d = nc.dram_tensor("self_m01", [T, S], BF16, kind="ExternalInput")
        sm01_d = smask_d
    gb_d = {}
    for key, shape in (("gb1", [P, 2, D]), ("gb2", [P, 2, D]), ("b2", [P, D])):
        if cfg[key]:
            gb_d[key] = nc.dram_tensor(key + "_t", shape, BF16, kind="ExternalInput")
    if mode == "body":
        out_d = nc.dram_tensor("out", [T, D], BF16, kind="ExternalOutput")
    else:
        # fc weights pre-tiled for group streaming: [NG, P, KT, VG]
        fcw_d = nc.dram_tensor("fcw", [NG, P, KT, VG], BF16, kind="ExternalInput")
        out_d = nc.dram_tensor("out", [T, V], BF16, kind="ExternalOutput")

    with tile.TileContext(nc) as tc:
        with (
            tc.tile_pool(name="wp", bufs=1) as wp,
            tc.tile_pool(name="big", bufs=10) as big,
            tc.tile_pool(name="hp", bufs=1) as hp,
            tc.tile_pool(name="wk", bufs=3) as wk,
            tc.tile_pool(name="st", bufs=8) as st,
            tc.tile_pool(name="fcx", bufs=2) as fcx,
            tc.tile_pool(name="lsp", bufs=4) as lsp,
        ):
            # --- tiny tensors first so the PE warmup + first matmuls can
            # start while the bulk weight loads stream in.
            ident_sb = wp.tile([P, P], BF16, tag="ident", name="ident")
            nc.sync.dma_start(ident_sb[:], id_d[:, :])
            biasp_sb = wp.tile([P, 32], F32, tag="biasp", name="biasp")
            nc.sync.dma_start(biasp_sb[:], biasp_d[:, :])
            eps_sb = wp.tile([P, 1], F32, tag="eps", name="eps")
            nc.vector.memset(eps_sb[:], EPS)

            def load3(handle, ktiles, free, dtype, name):
                t_ = wp.tile([P, ktiles, free], dtype, tag=name, name=name)
                nc.sync.dma_start(
                    t_[:], handle[:, :].rearrange("(k p) f -> p k f", p=P)
                )
                return t_

            # ordered by first use in the body
            body_sb = {}
            body_sb["x0T"] = load3(x0T_d, KT, T, BF16, "x0T")
            for nm in ("wq1", "wk1"):
                body_sb[nm] = load3(w_d[nm], KT, D, BF16, nm)
            body_sb["x0"] = load3(x0_d, TT, D, BF16, "x0")
            body_sb["wv1"] = load3(w_d["wv1"], KT, D, BF16, "wv1")
            smask_sb = cbias_sb = cm01_sb = None
            if cfg["skip_max"]:
                body_sb["sm01"] = load3(sm01_d, TT, S, BF16, "sm01")
                cm01_sb = wp.tile([P, S], BF16, tag="cm01", name="cm01")
                nc.sync.dma_start(cm01_sb[:], cm01_d[:, :])
            else:
                body_sb["sbias"] = load3(sbias_d, TT, S, BF16, "sbias")
                if cfg["exact_mask"]:
                    smask_sb = load3(smask_d, TT, S, BF16, "smask")
                cbias_sb = wp.tile([P, S], BF16, tag="cbias", name="cbias")
                nc.sync.dma_start(cbias_sb[:], cbias_d[:, :])
            bvrep_sb = wp.tile([P, 2, D], BF16, tag="bvrep", name="bvrep")
            nc.sync.dma_start(bvrep_sb[:], bvrep_d[:, :, :])
            body_sb["encT"] = load3(encT_d, KT, S, BF16, "encT")
            for nm in ("wq2", "wk2", "wv2"):
                body_sb[nm] = load3(w_d[nm], KT, D, BF16, nm)
            body_sb["w1"] = load3(w1_d, KT, 4 * D, BF16, "w1")
            body_sb["w2"] = load3(w2_d, JT, D, BF16, "w2")
            gb_sb = {}
            for key, t_ in gb_d.items():
                sh = [P, 2, D] if key in ("gb1", "gb2") else [P, D]
                gb_sb[key] = wp.tile(sh, BF16, tag=key, name=key)
                nc.sync.dma_start(gb_sb[key][:], t_[(slice(None),) * len(sh)])

            def mm_accum(psum, lhsT_list, rhs_list):
                n = len(lhsT_list)
                for i, (l_, r_) in enumerate(zip(lhsT_list, rhs_list)):
                    nc.tensor.matmul(psum, l_, r_, start=(i == 0), stop=(i == n - 1))

            def softmax_tile(ps_scores, bias_ap, m01_ap, attn_ap, scale_ap=1.0):
                den = st.tile([P, 1], F32, tag="den", name="den")
                rden = st.tile([P, 1], F32, tag="rden", name="rden")
                if cfg["skip_max"]:
                    # scores are O(1) and no row is fully masked: exp straight
                    # from PSUM (exp(-inf) never occurs; masking is the 0/1
                    # multiply afterwards, since exp(s + b01*NEG) = exp(s)*m01).
                    # scale_ap carries the lazily-applied LN rstd of the query.
                    nc.scalar.activation(attn_ap, ps_scores, Act.Exp, scale=scale_ap)
                    nc.vector.tensor_mul(attn_ap, attn_ap, m01_ap)
                else:
                    masked = wk.tile([P, S], F32, tag="masked", name="masked")
                    if m01_ap is not None:
                        tmp = wk.tile([P, S], F32, tag="masktmp", name="masktmp")
                        nc.vector.tensor_mul(tmp[:], ps_scores, m01_ap)
                        src = tmp[:]
                    else:
                        src = ps_scores
                    nc.vector.tensor_add(masked[:], src, bias_ap)
                    negmx = st.tile([P, 1], F32, tag="negmx", name="negmx")
                    nc.vector.reduce_max(
                        negmx[:], masked[:], axis=mybir.AxisListType.X, negate=True
                    )
                    nc.scalar.activation(
                        attn_ap, masked[:], Act.Exp, bias=negmx[:], scale=1.0
                    )
                nc.vector.reduce_sum(den[:], attn_ap, axis=mybir.AxisListType.X)
                nc.vector.reciprocal(rden[:], den[:])
                nc.vector.tensor_scalar_mul(attn_ap, attn_ap, rden[:])

            def ln_stats(xpre_ap, rstd_tag):
                # mean/var in one DVE pass via bn_stats -> (negmean, rstd)
                stats = st.tile([P, 6], F32, tag="bnst", name="bnst")
                nc.vector.bn_stats(stats[:], xpre_ap)
                aggr = st.tile([P, 2], F32, tag="bnag", name="bnag")
                nc.vector.bn_aggr(aggr[:], stats[:])
                negmean = st.tile([P, 1], F32, tag="negmean", name="negmean")
                nc.vector.tensor_scalar_mul(negmean[:], aggr[:, 0:1], -1.0)
                std = st.tile([P, 1], F32, tag="std", name="std")
                nc.scalar.activation(
                    std[:], aggr[:, 1:2], Act.Sqrt, bias=eps_sb[:], scale=1.0
                )
                rstd = st.tile([P, 1], F32, tag=rstd_tag, name=rstd_tag, bufs=4)
                nc.vector.reciprocal(rstd[:], std[:])
                return negmean, rstd

            def layer_norm(ps_in, res_ap, xn_ap, gb_key):
                xpre = wk.tile([P, D], F32, tag="xpre", name="xpre")
                nc.vector.tensor_add(xpre[:], ps_in, res_ap)
                negmean, rstd = ln_stats(xpre[:], "rstd")
                nc.vector.tensor_scalar(
                    xn_ap, xpre[:], negmean[:], rstd[:],
                    op0=Alu.add, op1=Alu.mult,
                )
                if gb_key is not None and cfg[gb_key]:
                    g_ = gb_sb[gb_key]
                    nc.any.tensor_mul(xn_ap, xn_ap, g_[:, 0, :])
                    nc.any.tensor_add(xn_ap, xn_ap, g_[:, 1, :])

            def ln_add(ps_in, res_ap, xpre_ap, pre_scale=None):
                # Lazy layer norm, part 1: just the residual add (optionally
                # scaling the psum by a deferred row factor first).  The
                # xpre (bf16, UN-normalized) feeds the next transposes —
                # downstream matmuls use mean-centered weights.
                if pre_scale is not None:
                    tmp = wk.tile([P, D], F32, tag="psc", name="psc")
                    nc.vector.tensor_scalar_mul(tmp[:], ps_in, pre_scale[:])
                    nc.vector.tensor_add(xpre_ap, tmp[:], res_ap)
                else:
                    nc.vector.tensor_add(xpre_ap, ps_in, res_ap)

            def ln_tail(xpre_ap, xn_ap, rstd_tag):
                # part 2 (emitted AFTER the transposes so the ACT-queue sqrt
                # never head-of-line blocks the transpose copies): stats,
                # rstd, and the off-critical-path normalized value.
                negmean, rstd = ln_stats(xpre_ap, rstd_tag)
                if xn_ap is not None:
                    nc.vector.tensor_scalar(
                        xn_ap, xpre_ap, negmean[:], rstd[:],
                        op0=Alu.add, op1=Alu.mult,
                    )
                return rstd

            with (
                tc.tile_pool(name="ps", bufs=4, space="PSUM") as ps_pool,
                tc.tile_pool(name="pst", bufs=3, space="PSUM") as pst_pool,
            ):
                # --- PE warmup: ~3.4us of identity matmuls so the HAM clock
                # gate reaches 8/8 before the real body matmuls arrive.  The
                # chain feeds the transpose identity, so it cannot be DCE'd:
                # ident2 = (32 * I) * (1/32) == I exactly.
                wm_ps = pst_pool.tile([P, P], F32, tag="wmps", name="wmps", bufs=1)
                for i in range(32):
                    nc.tensor.matmul(
                        wm_ps[:], ident_sb[:], ident_sb[:],
                        start=(i == 0), stop=(i == 31),
                    )
                ident2 = wp.tile([P, P], BF16, tag="ident2", name="ident2")
                nc.vector.tensor_scalar_mul(ident2[:], wm_ps[:], 1.0 / 32.0)

                def project_T(wtile, xtile, name, bcol):
                    # out[:, m, :] over d'-tiles; out[d', t] = sum_d w[d, d'] x[d, t]
                    o_ = big.tile([P, KT, T], BF16, tag="big", name=name)
                    for m in range(KT):
                        psum = ps_pool.tile([P, T], F32, tag="ps", name="ps")
                        mm_accum(
                            psum[:],
                            [wtile[:, k, m * P:(m + 1) * P] for k in range(KT)],
                            [xtile[:, k, :] for k in range(KT)],
                        )
                        nc.any.tensor_scalar_add(
                            o_[:, m, :], psum[:], biasp_sb[:, bcol + m:bcol + m + 1]
                        )
                    return o_

                def project_V(wtile, xtile, name, bv_idx):
                    # out[s, d'] = sum_d x[d, s] * w[d, d']
                    o_ = big.tile([P, TT, D], BF16, tag="big", name=name)
                    for m in range(TT):
                        psum = ps_pool.tile([P, D], F32, tag="ps", name="ps")
                        mm_accum(
                            psum[:],
                            [xtile[:, k, m * P:(m + 1) * P] for k in range(KT)],
                            [wtile[:, k, :] for k in range(KT)],
                        )
                        nc.any.tensor_add(o_[:, m, :], psum[:], bvrep_sb[:, bv_idx, :])
                    return o_

                def transpose_tile(src_ap, dst_ap, par):
                    pst = pst_pool.tile([P, P], BF16, tag="pst", name="pst")
                    nc.tensor.transpose(pst[:], src_ap, ident2[:])
                    if par % 2:
                        nc.scalar.copy(dst_ap, pst[:])
                    else:
                        nc.vector.tensor_copy(dst_ap, pst[:])

                def transpose_m(src, dst, a_, nFree):
                    # transpose the four 128x128 blocks of src m-tile a_ into
                    # dst column-slice a_
                    for b_ in range(nFree):
                        transpose_tile(
                            src[:, a_, b_ * P:(b_ + 1) * P],
                            dst[:, b_, a_ * P:(a_ + 1) * P],
                            a_ + b_,
                        )

                def transpose512(src, nPart, nFree, name):
                    dst = big.tile([P, nFree, nPart * P], BF16, tag="big", name=name)
                    for a_ in range(nPart):
                        transpose_m(src, dst, a_, nFree)
                    return dst

                def self_softmax_args(m):
                    if cfg["skip_max"]:
                        return None, body_sb["sm01"][:, m, :]
                    return body_sb["sbias"][:, m, :], (
                        smask_sb[:, m, :] if cfg["exact_mask"] else None
                    )

                # ---- self attention
                qt = project_T(body_sb["wq1"], body_sb["x0T"], "qt", 0)
                kt1 = project_T(body_sb["wk1"], body_sb["x0T"], "kt1", 4)
                v1 = project_V(body_sb["wv1"], body_sb["x0T"], "v1", 0)
                attn1 = big.tile([P, TT, S], BF16, tag="big", name="attn1")
                for m in range(TT):
                    pss = ps_pool.tile([P, S], F32, tag="ps", name="ps")
                    mm_accum(
                        pss[:],
                        [qt[:, k, m * P:(m + 1) * P] for k in range(KT)],
                        [kt1[:, k, :] for k in range(KT)],
                    )
                    bias_ap, m01_ap = self_softmax_args(m)
                    softmax_tile(pss[:], bias_ap, m01_ap, attn1[:, m, :])
                # encoder-side projections are independent of the softmax
                # chain above — emit them here so the PE has work while the
                # DVE/ACT softmax pipeline drains.
                k2t = project_T(body_sb["wk2"], body_sb["encT"], "k2t", 12)
                v2 = project_V(body_sb["wv2"], body_sb["encT"], "v2", 1)
                lazy = cfg["lazy"]
                # per-m: transpose attn tile m, attn@V, residual add; in lazy
                # mode the xpre transposes depend only on the add, so they
                # interleave here too — keeping the PE dense while the
                # stats/normalize chains drain on DVE/ACT.
                attn1T = big.tile([P, TT, T], BF16, tag="big", name="attn1T")
                x1s = big.tile([P, TT, D], BF16, tag="big", name="x1s")
                xn1 = big.tile([P, TT, D], BF16, tag="big", name="xn1") if lazy \
                    else x1s
                x1T = big.tile([P, KT, T], BF16, tag="big", name="x1T")
                rstd1 = []
                for m in range(TT):
                    transpose_m(attn1, attn1T, m, TT)
                    pso = ps_pool.tile([P, D], F32, tag="ps", name="ps")
                    mm_accum(
                        pso[:],
                        [attn1T[:, s_, m * P:(m + 1) * P] for s_ in range(TT)],
                        [v1[:, s_, :] for s_ in range(TT)],
                    )
                    if lazy:
                        ln_add(pso[:], body_sb["x0"][:, m, :], x1s[:, m, :])
                        transpose_m(x1s, x1T, m, KT)
                        rstd1.append(ln_tail(
                            x1s[:, m, :], xn1[:, m, :], "rstd1"))
                    else:
                        layer_norm(pso[:], body_sb["x0"][:, m, :],
                                   x1s[:, m, :], "gb1")
                if not lazy:
                    for m in range(TT):
                        transpose_m(x1s, x1T, m, KT)

                # ---- cross attention
                q2t = project_T(body_sb["wq2"], x1T, "q2t", 8)
                attn2 = big.tile([P, TT, S], BF16, tag="big", name="attn2")
                for m in range(TT):
                    pss = ps_pool.tile([P, S], F32, tag="ps", name="ps")
                    mm_accum(
                        pss[:],
                        [q2t[:, k, m * P:(m + 1) * P] for k in range(KT)],
                        [k2t[:, k, :] for k in range(KT)],
                    )
                    scl = rstd1[m][:] if lazy else 1.0
                    if cfg["skip_max"]:
                        softmax_tile(pss[:], None, cm01_sb[:, :], attn2[:, m, :],
                                     scale_ap=scl)
                    else:
                        softmax_tile(pss[:], cbias_sb[:, :], None, attn2[:, m, :])
                attn2T = big.tile([P, TT, T], BF16, tag="big", name="attn2T")
                x2s = big.tile([P, TT, D], BF16, tag="big", name="x2s")
                xn2 = big.tile([P, TT, D], BF16, tag="big", name="xn2") if lazy \
                    else x2s
                x2T = big.tile([P, KT, T], BF16, tag="big", name="x2T")
                rstd2 = []
                for m in range(TT):
                    transpose_m(attn2, attn2T, m, TT)
                    pso = ps_pool.tile([P, D], F32, tag="ps", name="ps")
                    mm_accum(
                        pso[:],
                        [attn2T[:, s_, m * P:(m + 1) * P] for s_ in range(TT)],
                        [v2[:, s_, :] for s_ in range(TT)],
                    )
                    if lazy:
                        ln_add(pso[:], xn1[:, m, :], x2s[:, m, :])
                        transpose_m(x2s, x2T, m, KT)
                        rstd2.append(ln_tail(
                            x2s[:, m, :], xn2[:, m, :], "rstd2"))
                    else:
                        layer_norm(pso[:], x1s[:, m, :], x2s[:, m, :], "gb2")
                if not lazy:
                    for m in range(TT):
                        transpose_m(x2s, x2T, m, KT)

                # ---- FFN
                if cfg["b2"]:
                    x2r = big.tile([P, TT, D], BF16, tag="big", name="x2r")
                    for m in range(TT):
                        nc.any.tensor_add(x2r[:, m, :], xn2[:, m, :],
                                          gb_sb["b2"][:, :])
                else:
                    x2r = xn2
                hT = hp.tile([P, JT, T], BF16, tag="hT", name="hT")
                for j in range(JT):
                    psh = ps_pool.tile([P, T], F32, tag="ps", name="ps")
                    mm_accum(
                        psh[:],
                        [body_sb["w1"][:, k, j * P:(j + 1) * P] for k in range(KT)],
                        [x2T[:, k, :] for k in range(KT)],
                    )
                    if not cfg["b1"]:
                        nc.any.tensor_scalar_max(hT[:, j, :], psh[:], 0.0)
                    else:
                        hb = wk.tile([P, T], F32, tag="hb", name="hb")
                        nc.any.tensor_scalar_add(
                            hb[:], psh[:], biasp_sb[:, 16 + j:16 + j + 1]
                        )
                        nc.any.tensor_scalar_max(hT[:, j, :], hb[:], 0.0)
                x3s = big.tile([P, TT, D], BF16, tag="big", name="x3s")
                x3T = None
                if mode != "body":
                    x3T = big.tile([P, KT, T], BF16, tag="big", name="x3T")
                rstd3 = []
                for m in range(TT):
                    psy = ps_pool.tile([P, D], F32, tag="ps", name="ps")
                    mm_accum(
                        psy[:],
                        [hT[:, j, m * P:(m + 1) * P] for j in range(JT)],
                        [body_sb["w2"][:, j, :] for j in range(JT)],
                    )
                    if lazy:
                        # relu output was computed from un-normalized x2pre:
                        # scale the FFN2 psum by rstd2 (relu is positively
                        # homogeneous) before adding the xn2 residual.
                        if mode == "body":
                            xpre3_ap = wk.tile(
                                [P, D], BF16, tag="xp3", name="xp3")[:]
                            xn3_ap = x3s[:, m, :]
                        else:
                            xpre3_ap = x3s[:, m, :]
                            xn3_ap = None
                        ln_add(psy[:], x2r[:, m, :], xpre3_ap,
                               pre_scale=rstd2[m])
                        if x3T is not None:
                            transpose_m(x3s, x3T, m, KT)
                        rstd3.append(ln_tail(xpre3_ap, xn3_ap, "rstd3"))
                    else:
                        layer_norm(psy[:], x2r[:, m, :], x3s[:, m, :], None)
                        if x3T is not None:
                            transpose_m(x3s, x3T, m, KT)
                xn3 = x3s

            if mode == "body":
                nc.sync.dma_start(
                    out_d[:, :].rearrange("(m p) d -> p m d", p=P), xn3[:]
                )
            else:
                # ---- fc: stream fcw groups, x3T stationary.
                # psum tile = 4 full banks [P, 4, 512]; all 2048 cols of a
                # group evacuate as one contiguous copy + one DMA.
                with tc.tile_pool(name="psfc", bufs=2, space="PSUM") as psfc_pool:
                    for g in range(NG):
                        fcg = fcx.tile([P, KT, VG], BF16, tag="fcg", name="fcg")
                        nc.sync.dma_start(fcg[:], fcw_d[g, :, :, :])
                        for m in range(TT):
                            psl = psfc_pool.tile(
                                [P, 4, 512], F32, tag="psl", name="psl"
                            )
                            for k in range(KT):
                                for v in range(4):
                                    nc.tensor.matmul(
                                        psl[:, v, 0:VCH],
                                        x3T[:, k, m * P:(m + 1) * P],
                                        fcg[:, k, v * VCH:(v + 1) * VCH],
                                        start=(k == 0), stop=(k == KT - 1),
                                    )
                            # evacuate as two halves on both DVE and ACT to
                            # halve the drain latency and balance the engines;
                            # in lazy mode this also applies the LN3 rstd.
                            lsb = lsp.tile([P, VG], BF16, tag="lsb", name="lsb")
                            if cfg["lazy"]:
                                nc.vector.tensor_scalar_mul(
                                    lsb[:, 0:2 * VCH], psl[:, 0:2, 0:VCH],
                                    rstd3[m][:],
                                )
                                nc.scalar.mul(
                                    lsb[:, 2 * VCH:VG], psl[:, 2:4, 0:VCH],
                                    rstd3[m][:],
                                )
                            else:
                                nc.vector.tensor_copy(
                                    lsb[:, 0:2 * VCH], psl[:, 0:2, 0:VCH]
                                )
                                nc.scalar.copy(
                                    lsb[:, 2 * VCH:VG], psl[:, 2:4, 0:VCH]
                                )
                            nc.sync.dma_start(
                                out_d[m * P:(m + 1) * P, g * VG:(g + 1) * VG],
                                lsb[:],
                            )

    nc.compile()
    return nc


def _host_prep(inputs):
    """Shared host-side prep: returns (cfg, in_maps)."""
    trg = np.asarray(inputs["trg_input"])
    enc = _f32(inputs["encoder_hiddens"])
    src_len = np.asarray(inputs["src_lengths"])
    emb = _f32(inputs["embedding"])
    g = {k: _f32(inputs[k]) for k in (
        "wq1", "bq1", "wk1", "bk1", "wv1", "bv1",
        "wq2", "bq2", "wk2", "bk2", "wv2", "bv2",
        "w1", "b1", "w2", "b2", "fcw", "fcb",
        "g1", "be1", "g2", "be2", "g3", "be3")}

    scale = 1.0 / math.sqrt(float(D))
    pe = _sinusoidal_pe(T, D)
    x0 = emb[trg] + pe[None]                      # [B, T, D] f32

    causal = np.tril(np.ones((T, T), dtype=bool))
    pad = trg != 0                                 # [B, T]
    self_mask = pad[:, None, :] & causal[None]     # [B, T, T]
    self_bias = np.where(self_mask, 0.0, NEG).astype(np.float32)
    exact_mask = bool((~self_mask).all(axis=2).any())

    sidx = np.arange(S)[None, :] < src_len[:, None]   # [B, S]
    cross_bias = np.where(sidx, 0.0, NEG).astype(np.float32)

    fcw_eff = g["g3"][:, None] * g["fcw"]
    fcb_eff = g["be3"] @ g["fcw"] + g["fcb"]

    cfg = {
        "exact_mask": exact_mask,
        "skip_max": not exact_mask,
        "b1": bool((g["b1"] != 0.0).any()),
        "gb1": bool((g["g1"] != 1.0).any() or (g["be1"] != 0.0).any()),
        "gb2": bool((g["g2"] != 1.0).any() or (g["be2"] != 0.0).any()),
        "b2": bool((g["b2"] != 0.0).any()),
    }
    # lazy layer-norm folding: mean subtraction is folded into column-centered
    # downstream weights; rstd is applied as a late row scale.  Requires the
    # plain-softmax path and no LN gains / q2 / ffn1 biases.
    cfg["lazy"] = bool(
        cfg["skip_max"] and not cfg["b1"] and not cfg["gb1"] and not cfg["gb2"]
        and not (g["bq2"] != 0.0).any()
    )
    if LAZY_OVERRIDE is not None:
        cfg["lazy"] = bool(LAZY_OVERRIDE) and cfg["lazy"]

    bias_p = np.zeros((P, 32), dtype=np.float32)
    bias_p[:, 0:4] = (g["bq1"] * scale).reshape(KT, P).T
    bias_p[:, 4:8] = g["bk1"].reshape(KT, P).T
    bias_p[:, 8:12] = (g["bq2"] * scale).reshape(KT, P).T
    bias_p[:, 12:16] = g["bk2"].reshape(KT, P).T
    bias_p[:, 16:32] = g["b1"].reshape(JT, P).T
    bias_v = np.stack(
        [np.broadcast_to(g["bv1"], (P, D)), np.broadcast_to(g["bv2"], (P, D))],
        axis=1,
    )

    wq2_eff = g["wq2"] * scale
    w1_eff = g["w1"]
    if cfg["lazy"]:
        # center the columns: x_centered @ w == x @ (w - colmean(w)) for
        # rows x of any mean, because sum_d (x_d - mu) w_d = x@(w - mean)
        # + mu*0 ... exactly folds the LN mean subtraction into the weight.
        wq2_eff = wq2_eff - wq2_eff.mean(axis=0, keepdims=True)
        w1_eff = w1_eff - w1_eff.mean(axis=0, keepdims=True)
        fcw_eff = fcw_eff - fcw_eff.mean(axis=0, keepdims=True)

    # fcw pre-tiled for streaming: [NG, P, KT, VG]
    fcw_t = _bf(fcw_eff.reshape(KT, P, NG, VG).transpose(2, 1, 0, 3))

    shared = {
        "wq1": _bf(g["wq1"] * scale), "wk1": _bf(g["wk1"]), "wv1": _bf(g["wv1"]),
        "wq2": _bf(wq2_eff), "wk2": _bf(g["wk2"]), "wv2": _bf(g["wv2"]),
        "w1": _bf(w1_eff), "w2": _bf(g["w2"]),
        "ident": _bf(np.eye(P, dtype=np.float32)),
        "bias_p": bias_p, "bias_v": _bf(bias_v),
        "fcw": fcw_t,
    }
    if cfg["gb1"]:
        shared["gb1_t"] = _bf(np.stack(
            [np.broadcast_to(g["g1"], (P, D)), np.broadcast_to(g["be1"], (P, D))], 1))
    if cfg["gb2"]:
        shared["gb2_t"] = _bf(np.stack(
            [np.broadcast_to(g["g2"], (P, D)), np.broadcast_to(g["be2"], (P, D))], 1))
    if cfg["b2"]:
        shared["b2_t"] = _bf(np.broadcast_to(g["b2"], (P, D)))

    in_maps = []
    for c in range(N_CORES):
        m = dict(shared)
        m["x0"] = _bf(x0[c])
        m["x0T"] = _bf(x0[c].T)
        m["encT"] = _bf(enc[c].T)
        if cfg["skip_max"]:
            m["self_m01"] = _bf(self_mask[c].astype(np.float32))
            m["cross_m01"] = _bf(
                np.broadcast_to(sidx[c].astype(np.float32), (P, S)))
        else:
            m["self_bias"] = _bf(self_bias[c])
            m["cross_bias"] = _bf(np.broadcast_to(cross_bias[c], (P, S)))
            if cfg["exact_mask"]:
                m["self_m01"] = _bf(self_mask[c].astype(np.float32))
        in_maps.append(m)
    return cfg, in_maps, fcb_eff


def _filter_in_maps(nc, in_maps):
    """Keep only the dram parameters this graph actually declares."""
    import concourse.mybir as mybir

    declared = set()
    for alloc in nc.m.functions[0].allocations:
        if isinstance(alloc, mybir.MemoryLocationSet) and alloc.kind == "ExternalInput":
            declared.add(alloc.memorylocations[0].name)
    return [{k: v for k, v in m.items() if k in declared} for m in in_maps]


def _run(nc, in_maps):
    global LAST_EXEC_NS, LAST_RESULTS, LAST_TMPDIR
    from concourse import bass_utils

    # Warm up the PJRT backend with a trivial op first — the bass custom-call
    # as the very first program has been observed to stall device init.
    import jax
    import jax.numpy as jnp

    jnp.add(
        jax.device_put(np.ones((8, 8), np.float32), jax.devices()[0]), 1.0
    ).block_until_ready()

    kwargs = {}
    if TRACE:
        _install_ntff_hook()
        kwargs = {"trace": True}
        if TRACE_DIR:
            import tempfile

            kwargs["tmpdir"] = tempfile.mkdtemp(prefix="run_", dir=TRACE_DIR)
            LAST_TMPDIR = kwargs["tmpdir"]
    res = bass_utils.run_bass_kernel_spmd(
        nc, _filter_in_maps(nc, in_maps), core_ids=list(range(N_CORES)), **kwargs
    )
    LAST_EXEC_NS = res.exec_time_ns
    LAST_RESULTS = res
    return res


def kernel(**inputs):
    cfg, in_maps, fcb_eff = _host_prep(inputs)
    key = (MODE,) + tuple(sorted(cfg.items()))
    if key not in _cache:
        _cache[key] = _build(cfg, MODE)
    nc = _cache[key]
    res = _run(nc, in_maps)
    if MODE != "full":
        return [np.asarray(res.results[c]["out"]) for c in range(N_CORES)]
    out = np.stack(
        [
            np.asarray(res.results[c]["out"]).astype(np.float32)
            for c in range(N_CORES)
        ],
        axis=0,
    )
    if fcb_eff.any():
        out += fcb_eff[None, None, :]
    return out
